# revision 42
# baseline (speedup 1.0000x reference)
"""Trainium2 Bass kernel for nn_MultiHeadCrossAttention (BS=4, S=512, DM=512, H=8).

Sharding: one attention head per NeuronCore (8 heads / 8 cores), host sums the
8 rank-64 output-projection partials (+ bo once).

v2 restructure vs baseline:
  * k-difference trick: rt_b[j,i] = 1/(1 + sum_{kb!=b} exp((k_kb-k_b)_j . q_b_i)).
    Only 12 exp tiles instead of 16, no numerator multiply, and the 6 unique
    k-diff stationaries serve 2 movings each (exp(+D) for batch u, exp(-D) for
    batch w via the activation scale).
  * P2 score matmuls use 64x64 array tiling: T(0,0) -> psum[0:64] and
    T(0,64) -> psum[64:128] run concurrently (contraction is only DK=64).
  * su = 1 + e1+e2+e3 accumulated on PE (identity matmuls + ones tile for the
    +1), reciprocal_approx_fast straight from PSUM into f32 rt (P3 runs f32).
  * P3: one exp per 2 i-chunks (N=512), e^2 on DVE (bf16 2x), contiguous
    batched Z/Q reductions writing Z_all/Q_all directly.
  * P6: bo added on host; evacuation batched N=1024.
"""

import numpy as np

BS, S, DM, H, DK = 4, 512, 512, 8, 64
EPS = 1e-6
NCORES = 8

PAIRS = [(0, 1), (0, 2), (0, 3), (1, 2), (1, 3), (2, 3)]


def build_program(nc, tile, mybir):
    from concourse.dve_ops import (
        RECIP_APPROX_FAST_CONSTS,
        RECIPROCAL_APPROX_FAST,
    )
    f32 = mybir.dt.float32
    bf16 = mybir.dt.bfloat16
    i32 = mybir.dt.int32
    AF = mybir.ActivationFunctionType
    OP = mybir.AluOpType
    AX = mybir.AxisListType

    qT_d = nc.dram_tensor("qT", [BS, 128, 4, S], bf16, kind="ExternalInput")
    kT_d = nc.dram_tensor("kT", [BS, 128, 4, S], bf16, kind="ExternalInput")
    vT_d = nc.dram_tensor("vT", [BS, 128, 4, S], bf16, kind="ExternalInput")
    Wq_d = nc.dram_tensor("Wq", [128, 4, DK], bf16, kind="ExternalInput")
    Wk_d = nc.dram_tensor("Wk", [128, 4, DK], bf16, kind="ExternalInput")
    Wv_d = nc.dram_tensor("Wv", [128, 4, DK], bf16, kind="ExternalInput")
    bqc_d = nc.dram_tensor("bqc", [DK, 1], f32, kind="ExternalInput")
    bkc_d = nc.dram_tensor("bkc", [DK, 1], f32, kind="ExternalInput")
    bv_d = nc.dram_tensor("bv", [1, DK], bf16, kind="ExternalInput")
    Wo_d = nc.dram_tensor("Wo", [DK, DM], bf16, kind="ExternalInput")
    al_d = nc.dram_tensor("alpha", [DK, 1], f32, kind="ExternalInput")
    b4_d = nc.dram_tensor("beta4", [DK, 1], f32, kind="ExternalInput")
    id_d = nc.dram_tensor("ident", [128, 128], bf16, kind="ExternalInput")
    outT_d = nc.dram_tensor("outT", [BS, DM, S], bf16, kind="ExternalOutput")

    with tile.TileContext(nc) as tc:
        with (
            tc.tile_pool(name="persist", bufs=1) as pp,
            tc.tile_pool(name="consts", bufs=1) as cp,
            tc.tile_pool(name="inp", bufs=3) as inp,
            tc.tile_pool(name="work", bufs=4) as wp,
            tc.tile_pool(name="vt", bufs=1) as vtp,
            tc.tile_pool(name="bwork", bufs=3) as bwp,
            tc.tile_pool(name="psum", bufs=1, space="PSUM") as psp,
        ):
            # ---- persistent SBUF ----
            qhT = pp.tile([DK, BS, S], bf16, tag="qhT")
            qh4 = pp.tile([DK, BS, S], bf16, tag="qh4")   # 4*qhT + beta4
            khT = pp.tile([DK, BS, S], bf16, tag="khT")
            dk = pp.tile([DK, 6, S], bf16, tag="dk")          # k-diff per pair
            vh_all = pp.tile([128, 4, BS, DK], bf16, tag="vh")  # [j128, jc, c, d]
            rt_all = pp.tile([128, BS, 4, S], bf16, tag="rt")   # [j128, b, jc, i]
            # exp tiles: [j128, b, jhalf, lane(3), jpsub(2), i]
            exg = pp.tile([128, BS, 2, 3, 2, S], bf16, tag="exg")
            e_all = pp.tile([128, BS, 4, BS * DK], bf16, tag="e")  # [i,b,ic,(c,d)]
            heads = pp.tile([DK, BS, S], bf16, tag="heads")
            Z_all = pp.tile([128, 64], f32, tag="Z")   # cols = b*16 + ic*4 + c
            Q_all = pp.tile([128, 64], f32, tag="Q")
            w1_all = pp.tile([128, 64], f32, tag="w1")
            w0_all = pp.tile([128, 16], bf16, tag="w0")
            w0T0 = pp.tile([8, 128], bf16, tag="w0T0")
            w0T1 = pp.tile([8, 128], bf16, tag="w0T1")
            w0f = pp.tile([1, 16 * 128], bf16, tag="w0f")

            Wq_s = cp.tile([128, 4, DK], bf16, tag="Wq")
            Wk_s = cp.tile([128, 4, DK], bf16, tag="Wk")
            Wv_s = cp.tile([128, 4, DK], bf16, tag="Wv")
            Wo_s = cp.tile([DK, DM], bf16, tag="Wo")
            bqc_s = cp.tile([DK, 1], f32, tag="bqc")
            bkc_s = cp.tile([DK, 1], f32, tag="bkc")
            bv_s = cp.tile([1, DK], bf16, tag="bv")
            al_s = cp.tile([DK, 1], f32, tag="al")
            b4_s = cp.tile([DK, 1], f32, tag="b4")
            id_s = cp.tile([128, 128], bf16, tag="id")
            ones = cp.tile([1, S], bf16, tag="ones")
            ones_f = cp.tile([1, S], f32, tag="ones_f")
            ones_b = cp.tile([128, S], bf16, tag="ones_b")  # +1 via PE id-matmul
            bvb = cp.tile([128, DK], f32, tag="bvb")

            # ---- input DMAs: k first (gates P1-k), then weights/consts ----
            ktiles = []
            kt0 = inp.tile([128, 4, S], bf16, tag="kt", name="kte0")
            nc.sync.dma_start(kt0[:], kT_d[0])
            ktiles.append(kt0)
            nc.sync.dma_start(Wk_s[:], Wk_d[:])
            nc.sync.dma_start(Wq_s[:], Wq_d[:])
            nc.sync.dma_start(bkc_s[:], bkc_d[:])
            nc.sync.dma_start(bqc_s[:], bqc_d[:])
            kt1 = inp.tile([128, 4, S], bf16, tag="kt", name="kte1")
            nc.sync.dma_start(kt1[:], kT_d[1])
            ktiles.append(kt1)
            # prefetch q batches 0-1 ahead of k batches 2-3 so the first
            # score-pair's deps (khT0/1, qhT0/1) land early
            qtiles = {}
            for b in range(2):
                qt = inp.tile([128, 4, S], bf16, tag="qt", name=f"qte{b}")
                nc.sync.dma_start(qt[:], qT_d[b])
                qtiles[b] = qt
            for b in range(2, BS):
                kt = inp.tile([128, 4, S], bf16, tag="kt", name=f"kte{b}")
                nc.sync.dma_start(kt[:], kT_d[b])
                ktiles.append(kt)
                qt = inp.tile([128, 4, S], bf16, tag="qt", name=f"qte{b}")
                nc.sync.dma_start(qt[:], qT_d[b])
                qtiles[b] = qt
            nc.sync.dma_start(Wv_s[:], Wv_d[:])
            nc.sync.dma_start(bv_s[:], bv_d[:])
            nc.sync.dma_start(id_s[:], id_d[:])
            nc.sync.dma_start(Wo_s[:], Wo_d[:])
            nc.sync.dma_start(al_s[:], al_d[:])
            nc.sync.dma_start(b4_s[:], b4_d[:])
            nc.vector.memset(ones[:], 1.0)
            nc.vector.memset(ones_f[:], 1.0)
            nc.vector.memset(ones_b[:], 1.0)
            # trigger the exp ACT table load immediately (overlaps input DMAs)
            nc.scalar.activation(ones_f[0:1, 0:8], ones_f[0:1, 0:8], AF.Exp)
            nc.vector.memset(ones_f[0:1, 0:8], 1.0)
            # PE warmup: junk matmuls (no DMA deps) so HAM un-throttles the
            # PE clock (4/8 -> 8/8) before the first real projection lands
            wps = psp.tile([128, S], f32, tag="su", bufs=2, name="warm")
            for _ in range(14):
                nc.tensor.matmul(wps[:], ones_b[:, 0:128], ones_b[:],
                                 start=True, stop=True)

            # PSUM tags: pe [128,2,512] x2bufs = 4 banks | su x2 = 2 | sc x2 = 2
            vtiles = []

            # ---- P1: k, q projections (khT/qhT at partitions 0-63) ----
            def emit_proj(W_s, b_c, dsrc, tag, dst, b, src=None):
                if src is None:
                    src = inp.tile([128, 4, S], bf16, tag=tag)
                    nc.sync.dma_start(src[:], dsrc[b])
                ps = psp.tile([DK, S], f32, tag="su", bufs=2, name="pproj")
                for mc in range(4):
                    nc.tensor.matmul(ps[:], W_s[:, mc, :], src[:, mc, :],
                                     start=(mc == 0), stop=(mc == 3))
                # evac on DVE (not ACT): keeps the ACT queue clear for the
                # pair-exp chain, which is the mid-kernel critical path
                nc.vector.tensor_scalar(dst[:, b, :], ps[:], 1.0, b_c[:],
                                        op0=OP.mult, op1=OP.add)

            def emit_dk(p):
                u, w = PAIRS[p]
                nc.vector.tensor_tensor(
                    dk[:, p, :], khT[:, w, :], khT[:, u, :], op=OP.subtract)

            with nc.named_scope("P1"):
                emit_proj(Wk_s, bkc_s, kT_d, "kt", khT, 0, src=ktiles[0])
                emit_proj(Wk_s, bkc_s, kT_d, "kt", khT, 1, src=ktiles[1])
                emit_proj(Wq_s, bqc_s, qT_d, "qt", qhT, 0, src=qtiles[0])
                emit_proj(Wq_s, bqc_s, qT_d, "qt", qhT, 1, src=qtiles[1])
                for b2 in range(BS):
                    vt = vtp.tile([128, 4, S], bf16, tag=f"vt{b2}")
                    nc.sync.dma_start(vt[:], vT_d[b2])
                    vtiles.append(vt)

            # ---- vh projection (f32, bias via broadcast matmul) ----
            def emit_vh():
                pb = psp.tile([128, S], f32, tag="sc", bufs=2, name="pbv")
                nc.tensor.matmul(pb[:, 0:DK], ones[:, 0:128], bv_s[:],
                                 start=True, stop=True)
                nc.vector.tensor_copy(bvb[:], pb[:, 0:DK])
                for c in range(BS):
                    vt = vtiles[c]
                    pv = psp.tile([128, 4, DK], f32, tag="sc", bufs=2, name="pv")
                    for jc in range(4):
                        for mc in range(4):
                            nc.tensor.matmul(
                                pv[:, jc, :], vt[:, mc, jc * 128:(jc + 1) * 128],
                                Wv_s[:, mc, :],
                                start=(mc == 0), stop=(mc == 3),
                            )
                    nc.vector.tensor_tensor(
                        vh_all[:, :, c, :], pv[:],
                        bvb[:].unsqueeze(1).broadcast_to((128, 4, DK)),
                        op=OP.add)

            # ---- P2: pair scores + exp; lane index bookkeeping ----
            # for batch b, lanes hold kb-terms in order of PAIRS traversal
            lane_of = {b: {} for b in range(BS)}
            for p, (u, w) in enumerate(PAIRS):
                lane_of[u][w] = len(lane_of[u])
                lane_of[w][u] = len(lane_of[w])

            def emit_pair(p, jh):
                u, w = PAIRS[p]
                psA = psp.tile([128, 2, S], f32, tag="pe", bufs=2, name="psA")
                psB = psp.tile([128, 2, S], f32, tag="pe", bufs=2, name="psB")
                for jpi in range(2):
                    sl = slice(jh * 256 + jpi * 128, jh * 256 + jpi * 128 + 128)
                    nc.tensor.matmul(psA[:, jpi, :], dk[:, p, sl],
                                     qhT[:, u, :], start=True, stop=True)
                    nc.tensor.matmul(psB[:, jpi, :], dk[:, p, sl],
                                     qhT[:, w, :], start=True, stop=True)
                nc.scalar.activation(
                    exg[:, u, jh, lane_of[u][w]], psA[:], AF.Exp)
                nc.scalar.activation(
                    exg[:, w, jh, lane_of[w][u]], psB[:], AF.Exp, scale=-1.0)

            def emit_tail(b, jh):
                # su = 1 + e0 + e1 + e2 per 128-j chunk, on PE; recip -> rt
                # (bf16 out via direct _custom_dve; wrapper asserts f32 out)
                rc = RECIP_APPROX_FAST_CONSTS
                for jpi in range(2):
                    su = psp.tile([128, S], f32, tag="su", bufs=2, name="ptail")
                    nc.tensor.matmul(su[:], id_s[:], ones_b[:],
                                     start=True, stop=False)
                    for l in range(3):
                        nc.tensor.matmul(su[:], id_s[:], exg[:, b, jh, l, jpi],
                                         start=False, stop=(l == 2))
                    nc.vector._custom_dve(
                        RECIPROCAL_APPROX_FAST,
                        out=rt_all[:, b, jh * 2 + jpi, :], in0=su[:],
                        s0=rc["s0"], s1=rc["s1"], imm2=rc["imm2"])

            # ---- P3: score matmuls (all-bf16 -> 1 cyc/row) + exp + Z/Q ----
            def emit_p3(b, icp):
                pc = psp.tile([128, 2, BS * DK], f32, tag="sc", bufs=2,
                              name="pp3")
                for ici in range(2):
                    ic = icp * 2 + ici
                    for jc in range(4):
                        nc.tensor.matmul(
                            pc[:, ici, :],
                            rt_all[:, b, jc, ic * 128:(ic + 1) * 128],
                            vh_all[:, jc].rearrange("p c d -> p (c d)"),
                            start=(jc == 0), stop=(jc == 3),
                        )
                esl = e_all[:, b, icp * 2:(icp + 1) * 2, :]
                nc.scalar.activation(esl, pc[:], AF.Exp)
                ev = esl.rearrange("p a (c d) -> p (a c) d", d=DK)
                col = b * 16 + icp * 8
                nc.vector.tensor_reduce(Z_all[:, col:col + 8], ev,
                                        axis=AX.X, op=OP.add)
                # e^2 on gpsimd (idle here) -> ACT chain stays pure exp
                e2t = wp.tile([128, 2, BS * DK], bf16, tag="e2")
                nc.gpsimd.tensor_tensor(e2t[:], esl, esl, op=OP.mult)
                nc.vector.tensor_reduce(
                    Q_all[:, col:col + 8],
                    e2t[:].rearrange("p a (c d) -> p (a c) d", d=DK),
                    axis=AX.X, op=OP.add)

            # ---- P4 stats (unchanged math from baseline) ----
            stp_cm = tc.tile_pool(name="stats", bufs=2)
            stp = stp_cm.__enter__()

            def emit_stats(h):
                SQ63 = float(np.sqrt(DK - 1.0))
                c0, c1 = h * 32, (h + 1) * 32
                Zs, Qs = Z_all[:, c0:c1], Q_all[:, c0:c1]
                t = stp.tile([128, 32], f32, tag="t", name="t")
                nc.vector.tensor_tensor(t[:], Zs, Zs, op=OP.mult)
                s = stp.tile([128, 32], f32, tag="s", name="s")
                nc.vector.scalar_tensor_tensor(
                    s[:], t[:], -1.0 / DK, Qs, op0=OP.mult, op1=OP.add)
                rinv = stp.tile([128, 32], f32, tag="rinv", name="rinv")
                nc.vector.reciprocal_approx_fast(rinv[:], t[:])
                v63 = stp.tile([128, 32], f32, tag="v63", name="v63")
                nc.vector.tensor_tensor(v63[:], s[:], rinv[:], op=OP.mult)
                # rsqrt(v63') via bit-trick seed + 1 Newton pass (all DVE —
                # ACT sqrt would thrash the activation table away from Exp)
                r_ = stp.tile([128, 32], f32, tag="r_", name="r_")
                nc.vector.tensor_scalar(r_[:].bitcast(i32), v63[:].bitcast(i32),
                                        1, None, op0=OP.logical_shift_right)
                nc.vector.tensor_scalar(r_[:].bitcast(i32), r_[:].bitcast(i32),
                                        -1, 0x5F3759DF, op0=OP.mult, op1=OP.add)
                nt = stp.tile([128, 32], f32, tag="nt", name="nt")
                nc.vector.tensor_tensor(nt[:], v63[:], r_[:], op=OP.mult)
                nc.vector.tensor_tensor(nt[:], nt[:], r_[:], op=OP.mult)
                nc.vector.tensor_scalar(nt[:], nt[:], -0.5, 1.5,
                                        op0=OP.mult, op1=OP.add)
                nc.vector.tensor_tensor(r_[:], r_[:], nt[:], op=OP.mult)
                g = stp.tile([128, 32], f32, tag="g", name="g")
                nc.vector.tensor_scalar(g[:], r_[:], SQ63, None, op0=OP.mult)
                zr = stp.tile([128, 32], f32, tag="zr", name="zr")
                nc.vector.reciprocal_approx_fast(zr[:], Zs)
                nc.vector.tensor_tensor(w1_all[:, c0:c1], g[:], zr[:],
                                        op=OP.mult)
                gs = stp.tile([128, 8], f32, tag="gs", name="gs")
                nc.vector.tensor_reduce(
                    gs[:], g[:].rearrange("p (s c) -> p s c", c=4), axis=AX.X,
                    op=OP.add)
                nc.vector.tensor_scalar(w0_all[:, h * 8:(h + 1) * 8], gs[:],
                                        -1.0 / DK, None, op0=OP.mult)
                pw = psp.tile([128, S], f32, tag="su", bufs=2, name="pw")
                nc.tensor.matmul(pw[:8, 0:128], w0_all[:, h * 8:(h + 1) * 8],
                                 id_s[:], start=True, stop=True)
                w0Th = w0T0 if h == 0 else w0T1
                nc.vector.tensor_copy(w0Th[:, :], pw[:8, 0:128])
                nc.sync.dma_start(
                    w0f[0:1, h * 1024:(h + 1) * 1024]
                    .rearrange("o (s f) -> o s f", s=8),
                    w0Th[:, :])

            # ---- P5 ----
            def emit_p5(b):
                bsc = bwp.tile([128, 4, 4, DK], f32, tag="bsc")
                w1b = (w1_all[:, b * 16:(b + 1) * 16]
                       .rearrange("p (i c) -> p i c", c=4)
                       .unsqueeze(-1).broadcast_to((128, 4, 4, DK)))
                nc.gpsimd.tensor_tensor(
                    bsc[:],
                    e_all[:, b, :, :].rearrange("p i (c d) -> p i c d", d=DK),
                    w1b, op=OP.mult,
                )
                t01 = bwp.tile([128, 4, DK], f32, tag="t01")
                t23 = bwp.tile([128, 4, DK], f32, tag="t23")
                nc.gpsimd.tensor_tensor(t01[:], bsc[:, :, 0, :],
                                        bsc[:, :, 1, :], op=OP.add)
                nc.vector.tensor_tensor(t23[:], bsc[:, :, 2, :],
                                        bsc[:, :, 3, :], op=OP.add)
                ball = bwp.tile([128, 4, DK], bf16, tag="ball")
                nc.vector.tensor_tensor(ball[:], t01[:], t23[:], op=OP.add)
                pbig = psp.tile([DK, S], f32, tag="sc", bufs=2, name="pbig")
                # w0 broadcast FIRST (single start=True sets has_written for
                # the whole bank; start=True mid-group would clear other
                # chunks' bits and turn accumulation into overwrite)
                nc.tensor.matmul(
                    pbig[:], ones[:, 0:DK],
                    w0f[0:1, b * 512:(b + 1) * 512],
                    start=True, stop=False,
                )
                # then accumulate transposed ball (regular bf16 mm, mov=id)
                for ic in range(4):
                    nc.tensor.matmul(pbig[:, ic * 128:(ic + 1) * 128],
                                     ball[:, ic, :], id_s[:],
                                     start=False, stop=(ic == 3))
                nc.vector.scalar_tensor_tensor(
                    heads[:, b, :], pbig[:], al_s[:], qh4[:, b, :],
                    op0=OP.mult, op1=OP.add,
                )

            # ---- P6 (bo added on host) ----
            def emit_p6(b):
                for nchp in range(2):
                    po = psp.tile([128, 2, S], f32, tag="pe", bufs=2, name="po")
                    for ni in range(2):
                        nch = nchp * 2 + ni
                        nc.tensor.matmul(
                            po[:, ni, :], Wo_s[:, nch * 128:(nch + 1) * 128],
                            heads[:, b, :], start=True, stop=True,
                        )
                    ot = bwp.tile([128, 2, S], bf16, tag="ot")
                    for ni in range(2):
                        nc.scalar.activation(ot[:, ni, :], po[:, ni, :],
                                             AF.Identity)
                        nch = nchp * 2 + ni
                        nc.sync.dma_start(
                            outT_d[b, nch * 128:(nch + 1) * 128, :],
                            ot[:, ni, :])

            # junk matmuls into a fresh pe-tag psum buf: fill PE-idle gaps in
            # the tail so HAM keeps the clock at 8/8
            def emit_warm(n=2):
                jp = psp.tile([128, 2, S], f32, tag="pe", bufs=2, name="junk")
                for _ in range(n):
                    nc.tensor.matmul(jp[:, 0, :], ones_b[:, 0:128], ones_b[:],
                                     start=True, stop=True)

            with nc.named_scope("P2356"):
                emit_dk(0)             # (0,1): khT0/1 + qhT0/1 ready
                for jh in range(2):
                    emit_pair(0, jh)   # (0,1)
                emit_proj(Wk_s, bkc_s, kT_d, "kt", khT, 2, src=ktiles[2])
                emit_proj(Wq_s, bqc_s, qT_d, "qt", qhT, 2, src=qtiles[2])
                emit_dk(1)
                for jh in range(2):
                    emit_pair(1, jh)   # (0,2): needs khT2 AND qhT2
                emit_proj(Wk_s, bkc_s, kT_d, "kt", khT, 3, src=ktiles[3])
                emit_proj(Wq_s, bqc_s, qT_d, "qt", qhT, 3, src=qtiles[3])
                emit_dk(2)
                for jh in range(2):
                    emit_pair(2, jh)   # (0,3): needs khT3 AND qhT3
                emit_dk(3)
                emit_dk(4)
                emit_dk(5)
                for b in range(BS):
                    nc.vector.tensor_scalar(qh4[:, b, :], qhT[:, b, :],
                                            4.0, b4_s[:], op0=OP.mult,
                                            op1=OP.add)
                emit_vh()              # vT DMAs have landed by now
                emit_tail(0, 0)
                emit_tail(0, 1)
                for jh in range(2):
                    emit_pair(3, jh)   # (1,2)
                emit_p3(0, 0)
                emit_p3(0, 1)
                for jh in range(2):
                    emit_pair(4, jh)   # (1,3)
                emit_tail(1, 0)
                emit_tail(1, 1)
                for jh in range(2):
                    emit_pair(5, jh)   # (2,3)
                emit_p3(1, 0)
                emit_p3(1, 1)
                emit_tail(2, 0)
                emit_tail(2, 1)
                emit_tail(3, 0)
                emit_tail(3, 1)
                emit_stats(0)
                emit_p3(2, 0)
                emit_p3(2, 1)
                emit_p3(3, 0)
                emit_p3(3, 1)
                emit_p5(0)
                emit_stats(1)
                emit_p5(1)
                emit_p6(0)
                emit_p5(2)
                emit_p6(1)
                emit_p5(3)
                emit_p6(2)
                emit_p6(3)
            stp_cm.__exit__(None, None, None)

    return nc


def _build():
    import concourse.bass as bass  # noqa
    import concourse.tile as tile
    from concourse import bacc, mybir

    nc = bacc.Bacc("TRN2", target_bir_lowering=False, debug=False,
                   num_devices=NCORES)
    build_program(nc, tile, mybir)
    nc.compile()
    return nc


_cached_nc = None
_bo_full = None


def make_in_maps(q, k, v, Wq, bq, Wk, bk, Wv, bv, Wo, bo, alpha, beta):
    import ml_dtypes
    bft = ml_dtypes.bfloat16

    def prelay(x):
        xT = np.swapaxes(np.asarray(x, np.float32), 1, 2)  # [B, DM, S]
        return np.ascontiguousarray(
            xT.reshape(BS, 4, 128, S).transpose(0, 2, 1, 3)).astype(bft)

    def wlay(W):  # [DM, DK] -> [128, 4, DK]
        return np.ascontiguousarray(
            np.asarray(W, np.float32).reshape(4, 128, DK).transpose(1, 0, 2)
        ).astype(bft)

    qT, kT, vT = prelay(q), prelay(k), prelay(v)
    Wq, Wk, Wv, Wo = (np.asarray(x, np.float32) for x in (Wq, Wk, Wv, Wo))
    bq, bk, bv, bo = (np.asarray(x, np.float32) for x in (bq, bk, bv, bo))
    alpha, beta = np.asarray(alpha, np.float32), np.asarray(beta, np.float32)
    ident = np.eye(128, dtype=ml_dtypes.bfloat16)
    scale = np.float32(1.0 / np.sqrt(np.float32(DK)))
    in_maps = []
    for h in range(NCORES):
        sl = slice(h * DK, (h + 1) * DK)
        in_maps.append({
            "qT": qT, "kT": kT, "vT": vT,
            "Wq": wlay(Wq[:, sl]),
            "Wk": wlay(Wk[:, sl]),
            "Wv": wlay(Wv[:, sl] * scale),
            "bqc": np.ascontiguousarray(bq[sl])[:, None].astype(np.float32),
            "bkc": np.ascontiguousarray(bk[sl])[:, None].astype(np.float32),
            "bv": np.ascontiguousarray(bv[sl] * scale)[None, :].astype(bft),
            "Wo": np.ascontiguousarray(Wo[sl, :]).astype(bft),
            "alpha": np.ascontiguousarray(alpha)[:, None],
            "beta4": np.ascontiguousarray(4.0 * beta)[:, None],
            "ident": ident,
        })
    return in_maps


def assemble(results, bo=None):
    out = np.zeros((BS, S, DM), np.float32)
    for r in results:
        out += np.swapaxes(np.asarray(r["outT"], np.float32), 1, 2)
    if bo is not None:
        out += np.asarray(bo, np.float32)[None, None, :]
    return out


def kernel(**inputs) -> np.ndarray:
    global _cached_nc
    from concourse.bass_utils import run_bass_kernel_spmd

    if _cached_nc is None:
        _cached_nc = _build()
    in_maps = make_in_maps(**inputs)
    res = run_bass_kernel_spmd(_cached_nc, in_maps, list(range(NCORES)))
    return assemble(res.results, bo=inputs["bo"])

